# revision 10
# baseline (speedup 1.0000x reference)
"""Trainium2 Bass kernel for nn_PointNetLayer (masked per-particle MLP).

Reference computation (per particle, 524288 of them):
    feats = events[..., :16]; mask = events[..., 16]
    h = relu(relu(relu(feats@W1+b1)@W2+b2)@W3+b3)
    out = concat(h, 1) * mask          # (..., 65)

Strategy (pure data parallelism over 8 cores, 65536 particles each):
  - Host pads particle rows 17 -> 32 floats; DMA in contiguous
    [128, 2048] chunks (SWDGE), 64 padded rows per partition.
  - PE transposes [128,128] windows (float32r, 1.5 cyc/row) putting 4
    particle-blocks at 32-aligned partition offsets (features on
    partitions, particles on free dim).
  - 3-layer MLP as block-diagonal-paired float32r matmuls (2 blocks of
    64 features in 128 partitions, free dim 512). Biases are applied in
    the PSUM->SBUF relu copies (per-partition bias APs).
  - PE transposes back to particle-major; mask applied during the
    PSUM->SBUF copy via a broadcast AP; mask column written separately.
  - DMA out contiguous [128, 4160] chunks (HWDGE).
"""

import sys

sys.path.insert(0, "/opt/trn_rl_repo")

import numpy as np

import concourse.bass as bass
import concourse.bacc as bacc
import concourse.mybir as mybir
import concourse.tile as tile
from concourse.ap import AP
from concourse.bass_utils import run_bass_kernel_spmd

F32 = mybir.dt.float32
F32R = mybir.dt.float32r
AF = mybir.ActivationFunctionType
ALU = mybir.AluOpType

B, P, F = 4096, 128, 17
DIN, H, DOUT = 16, 64, 64
NCORES = 8
NPART = B * P // NCORES          # 65536 particles per core
SUP = 2048                        # particles per super-tile (compute unit)
CH_SUP = 4                        # super-tiles per DMA chunk
CHUNK = SUP * CH_SUP              # 8192 particles per DMA
NCH = NPART // CHUNK              # 8 chunks
FOUT = DOUT + 1                   # 65
FP = 32                           # host-padded row length (17 -> 32)

TRACE = False
LAST_RESULTS = None

_CACHE = {}


def _inject(ap2d: AP, dims) -> AP:
    """Insert extra free dims between the partition dim and the last free
    dim of a 2-d AP."""
    lst = list(ap2d.ap)
    assert len(lst) == 2, lst
    return AP(ap2d.tensor, ap2d.offset, [lst[0], *[list(d) for d in dims], lst[1]])


def _build(weights):
    W1, b1, W2, b2, W3, b3 = weights
    nc = bacc.Bacc("TRN2", target_bir_lowering=False, debug=False,
                   num_devices=NCORES)

    ev = nc.dram_tensor("events", [NPART, FP], F32R, kind="ExternalInput")
    out = nc.dram_tensor("out", [NPART, FOUT], F32, kind="ExternalOutput")

    # Host-preprocessed constant layouts (embedded in the NEFF).
    W1P = np.zeros((128, 128), np.float32)
    for r, c in ((0, 0), (32, 64), (64, 0), (96, 64)):
        W1P[r:r + DIN, c:c + H] = W1
    W2P = np.zeros((128, 128), np.float32)
    W2P[0:64, 0:64] = W2
    W2P[64:128, 64:128] = W2
    W3P = np.zeros((128, 128), np.float32)
    W3P[0:64, 0:64] = W3
    W3P[64:128, 64:128] = W3
    I128 = np.eye(128, dtype=np.float32)
    b1b = np.concatenate([b1, b1])[:, None].astype(np.float32)
    b2b = np.concatenate([b2, b2])[:, None].astype(np.float32)
    b3b = np.concatenate([b3, b3])[:, None].astype(np.float32)

    w1d = nc.inline_tensor(W1P, "w1p")
    w2d = nc.inline_tensor(W2P, "w2p")
    w3d = nc.inline_tensor(W3P, "w3p")
    idd = nc.inline_tensor(I128, "i128")
    b1d = nc.inline_tensor(b1b, "b1b")
    b2d = nc.inline_tensor(b2b, "b2b")
    b3d = nc.inline_tensor(b3b, "b3b")

    with tile.TileContext(nc) as tc:
        _kernel_body(tc, ev, out, w1d, w2d, w3d, idd, b1d, b2d, b3d)

    nc.compile()
    return nc


def _kernel_body(tc, ev, out, w1d, w2d, w3d, idd, b1d, b2d, b3d):
    nc = tc.nc
    from contextlib import ExitStack
    ctx = ExitStack()

    cpool = ctx.enter_context(tc.tile_pool(name="consts", bufs=1))
    w1f = cpool.tile([128, 128], F32, name="w1f")
    w2f = cpool.tile([128, 128], F32, name="w2f")
    w3f = cpool.tile([128, 128], F32, name="w3f")
    i1f = cpool.tile([128, 128], F32, name="i1f")
    w1s = cpool.tile([128, 128], F32R, name="w1s")
    w2s = cpool.tile([128, 128], F32R, name="w2s")
    w3s = cpool.tile([128, 128], F32R, name="w3s")
    i128 = cpool.tile([128, 128], F32R, name="i128")
    b1s = cpool.tile([128, 1], F32, name="b1s")
    b2s = cpool.tile([128, 1], F32, name="b2s")
    b3s = cpool.tile([128, 1], F32, name="b3s")
    for t, d in ((w1f, w1d), (w2f, w2d), (w3f, w3d), (i1f, idd),
                 (b1s, b1d), (b2s, b2d), (b3s, b3d)):
        nc.sync.dma_start(t[:], d.ap())
    for dst, srct in ((w1s, w1f), (w2s, w2f), (w3s, w3f), (i128, i1f)):
        nc.vector.tensor_copy(dst[:], srct[:])

    xpool = ctx.enter_context(tc.tile_pool(name="xin", bufs=2))
    xtpool = ctx.enter_context(tc.tile_pool(name="xts", bufs=2))
    h1pool = ctx.enter_context(tc.tile_pool(name="h1s", bufs=2))
    h2pool = ctx.enter_context(tc.tile_pool(name="h2s", bufs=2))
    h3pool = ctx.enter_context(tc.tile_pool(name="h3s", bufs=2))
    opool = ctx.enter_context(tc.tile_pool(name="obuf", bufs=2))
    tinp = ctx.enter_context(tc.tile_pool(name="tinp", bufs=2, space="PSUM"))
    mmp = ctx.enter_context(tc.tile_pool(name="mmp", bufs=3, space="PSUM"))

    for c in range(NCH):
        # ---- load [128, 2048] contiguous (64 padded rows per partition)
        xin = xpool.tile([128, CH_SUP * 16 * FP], F32R, tag="xin")
        src = AP(ev, c * CHUNK * FP, [[CH_SUP * 16 * FP, 128],
                                      [1, CH_SUP * 16 * FP]])
        nc.gpsimd.dma_start(xin[:], src)
        obuf = opool.tile([128, CH_SUP * 16 * FOUT], F32, tag="obuf")

        for s in range(CH_SUP):
            xv = xin[:, 512 * s:512 * (s + 1)]
            ov = obuf[:, 1040 * s:1040 * (s + 1)]

            # ---- transpose in: 4 x ([128, 128] window -> [128, 128])
            tpsum = tinp.tile([128, 512], F32R, tag="tps")
            for t in range(4):
                nc.tensor.transpose(tpsum[:, 128 * t:128 * (t + 1)],
                                    xv[:, 128 * t:128 * (t + 1)], i128[:])

            xts = xtpool.tile([128, 512], F32R, tag="xts")
            nc.scalar.copy(xts[:], tpsum[:])

            # ---- L1: blockdiag pairs, K=49, N=512
            ps1 = mmp.tile([128, 1024], F32, tag="mm")
            nc.tensor.matmul(ps1[:, 0:512], w1s[0:49, :], xts[0:49, :],
                             start=True, stop=True)
            nc.tensor.matmul(ps1[:, 512:1024], w1s[64:113, :], xts[64:113, :],
                             start=True, stop=True)
            h1s = h1pool.tile([128, 1024], F32R, tag="h1s")
            nc.scalar.activation(h1s[:], ps1[:], AF.Relu, bias=b1s[:])

            # ---- L2
            ps2 = mmp.tile([128, 1024], F32, tag="mm")
            nc.tensor.matmul(ps2[:, 0:512], w2s[:], h1s[:, 0:512],
                             start=True, stop=True)
            nc.tensor.matmul(ps2[:, 512:1024], w2s[:], h1s[:, 512:1024],
                             start=True, stop=True)
            h2s = h2pool.tile([128, 1024], F32R, tag="h2s")
            nc.vector.tensor_scalar(h2s[:], ps2[:], b2s[:], 0.0,
                                    ALU.add, ALU.max)

            # ---- L3
            ps3 = mmp.tile([128, 1024], F32, tag="mm")
            nc.tensor.matmul(ps3[:, 0:512], w3s[:], h2s[:, 0:512],
                             start=True, stop=True)
            nc.tensor.matmul(ps3[:, 512:1024], w3s[:], h2s[:, 512:1024],
                             start=True, stop=True)
            h3s = h3pool.tile([128, 1024], F32R, tag="h3s")
            nc.scalar.activation(h3s[:], ps3[:], AF.Relu, bias=b3s[:])

            # ---- transpose out: 8 x [128,128] -> particle-major pairs
            psb = mmp.tile([128, 1024], F32R, tag="mm")
            for u in range(8):
                nc.tensor.transpose(psb[:, 128 * u:128 * (u + 1)],
                                    h3s[:, 128 * u:128 * (u + 1)], i128[:])

            # ---- masked copy into output buffer + mask column
            # column groups: half h=0 -> blocks (4t, 4t+1); h=1 -> (4t+2, 4t+3)
            for h in range(2):
                dst = _inject(ov[:, 130 * h:130 * h + 64], [[260, 4], [65, 2]])
                msk = xv[:, 16 + 64 * h:16 + 64 * h + 1]
                msk = AP(msk.tensor, msk.offset,
                         [list(msk.ap[0]), [128, 4], [32, 2], [0, 64]])
                nc.vector.tensor_tensor(dst, psb[:, 512 * h:512 * (h + 1)],
                                        msk, ALU.mult)
            mcol_dst = _inject(ov[:, 64:65], [[65, 16]])
            mcol_src = _inject(xv[:, 16:17], [[32, 16]])
            nc.vector.tensor_copy(mcol_dst, mcol_src)

        dst = AP(out, c * CHUNK * FOUT, [[CH_SUP * 16 * FOUT, 128],
                                         [1, CH_SUP * 16 * FOUT]])
        nc.sync.dma_start(dst, obuf[:])

    ctx.close()


def kernel(events, W1, b1, W2, b2, W3, b3):
    global LAST_RESULTS
    events = np.ascontiguousarray(np.asarray(events, dtype=np.float32))
    key = "nc"
    if key not in _CACHE:
        _CACHE[key] = _build((np.asarray(W1, np.float32), np.asarray(b1, np.float32),
                              np.asarray(W2, np.float32), np.asarray(b2, np.float32),
                              np.asarray(W3, np.float32), np.asarray(b3, np.float32)))
    nc = _CACHE[key]

    flat = events.reshape(B * P, F)
    padded = np.zeros((B * P, FP), dtype=np.float32)
    padded[:, :F] = flat
    in_maps = [{"events": np.ascontiguousarray(padded[c * NPART:(c + 1) * NPART])}
               for c in range(NCORES)]
    res = run_bass_kernel_spmd(nc, in_maps, core_ids=list(range(NCORES)),
                               trace=TRACE)
    LAST_RESULTS = res
    out = np.concatenate([res.results[c]["out"] for c in range(NCORES)], axis=0)
    return out.reshape(B, P, FOUT)


# revision 13
# speedup vs baseline: 480.1149x; 480.1149x over previous
"""Trainium2 Bass kernel for nn_PointNetLayer (masked per-particle MLP).

Reference computation (per particle, 524288 of them):
    feats = events[..., :16]; mask = events[..., 16]
    h = relu(relu(relu(feats@W1+b1)@W2+b2)@W3+b3)
    out = concat(h, 1) * mask          # (..., 65)

Strategy (pure data parallelism over 8 cores, 65536 particles each):
  - Host pads particle rows 17 -> 32 floats; DMA in contiguous
    [128, 2048] chunks (SWDGE), 64 padded rows per partition.
  - PE transposes [128,128] windows (float32r, 1.5 cyc/row) putting 4
    particle-blocks at 32-aligned partition offsets (features on
    partitions, particles on free dim).
  - 3-layer MLP as block-diagonal-paired float32r matmuls (2 blocks of
    64 features in 128 partitions, free dim 512). Biases are applied in
    the PSUM->SBUF relu copies (per-partition bias APs).
  - PE transposes back to particle-major; mask applied during the
    PSUM->SBUF copy via a broadcast AP; mask column written separately.
  - DMA out contiguous [128, 4160] chunks (HWDGE).
"""

import sys

sys.path.insert(0, "/opt/trn_rl_repo")

import numpy as np

import concourse.bass as bass
import concourse.bacc as bacc
import concourse.mybir as mybir
import concourse.tile as tile
from concourse.ap import AP
from concourse.bass_utils import run_bass_kernel_spmd

F32 = mybir.dt.float32
F32R = mybir.dt.float32r
AF = mybir.ActivationFunctionType
ALU = mybir.AluOpType

B, P, F = 4096, 128, 17
DIN, H, DOUT = 16, 64, 64
NCORES = 8
NPART = B * P // NCORES          # 65536 particles per core
SUP = 2048                        # particles per super-tile (compute unit)
CH_SUP = 4                        # super-tiles per DMA chunk
CHUNK = SUP * CH_SUP              # 8192 particles per DMA
NCH = NPART // CHUNK              # 8 chunks
FOUT = DOUT + 1                   # 65
FP = 32                           # host-padded row length (17 -> 32)

TRACE = False
LAST_RESULTS = None

_CACHE = {}


def _inject(ap2d: AP, dims) -> AP:
    """Insert extra free dims between the partition dim and the last free
    dim of a 2-d AP."""
    lst = list(ap2d.ap)
    assert len(lst) == 2, lst
    return AP(ap2d.tensor, ap2d.offset, [lst[0], *[list(d) for d in dims], lst[1]])


def _build(weights, reps=1):
    W1, b1, W2, b2, W3, b3 = weights
    nc = bacc.Bacc("TRN2", target_bir_lowering=False, debug=False,
                   num_devices=NCORES)

    ev = nc.dram_tensor("events", [NPART, FP], F32R, kind="ExternalInput")
    out = nc.dram_tensor("out", [NPART, FOUT], F32, kind="ExternalOutput")

    # Host-preprocessed constant layouts (embedded in the NEFF).
    W1P = np.zeros((128, 128), np.float32)
    for r, c in ((0, 0), (32, 64), (64, 0), (96, 64)):
        W1P[r:r + DIN, c:c + H] = W1
    W2P = np.zeros((128, 128), np.float32)
    W2P[0:64, 0:64] = W2
    W2P[64:128, 64:128] = W2
    W3P = np.zeros((128, 128), np.float32)
    W3P[0:64, 0:64] = W3
    W3P[64:128, 64:128] = W3
    I128 = np.eye(128, dtype=np.float32)
    b1b = np.concatenate([b1, b1])[:, None].astype(np.float32)
    b2b = np.concatenate([b2, b2])[:, None].astype(np.float32)
    b3b = np.concatenate([b3, b3])[:, None].astype(np.float32)

    w1d = nc.inline_tensor(W1P, "w1p")
    w2d = nc.inline_tensor(W2P, "w2p")
    w3d = nc.inline_tensor(W3P, "w3p")
    idd = nc.inline_tensor(I128, "i128")
    b1d = nc.inline_tensor(b1b, "b1b")
    b2d = nc.inline_tensor(b2b, "b2b")
    b3d = nc.inline_tensor(b3b, "b3b")

    with tile.TileContext(nc) as tc:
        _kernel_body(tc, ev, out, w1d, w2d, w3d, idd, b1d, b2d, b3d, reps)

    nc.compile()
    return nc


def _kernel_body(tc, ev, out, w1d, w2d, w3d, idd, b1d, b2d, b3d, reps=1):
    nc = tc.nc
    from contextlib import ExitStack
    ctx = ExitStack()

    cpool = ctx.enter_context(tc.tile_pool(name="consts", bufs=1))
    w1f = cpool.tile([128, 128], F32, name="w1f")
    w2f = cpool.tile([128, 128], F32, name="w2f")
    w3f = cpool.tile([128, 128], F32, name="w3f")
    i1f = cpool.tile([128, 128], F32, name="i1f")
    w1s = cpool.tile([128, 128], F32R, name="w1s")
    w2s = cpool.tile([128, 128], F32R, name="w2s")
    w3s = cpool.tile([128, 128], F32R, name="w3s")
    i128 = cpool.tile([128, 128], F32R, name="i128")
    b1s = cpool.tile([128, 1], F32, name="b1s")
    b2s = cpool.tile([128, 1], F32, name="b2s")
    b3s = cpool.tile([128, 1], F32, name="b3s")
    for t, d in ((w1f, w1d), (w2f, w2d), (w3f, w3d), (i1f, idd),
                 (b1s, b1d), (b2s, b2d), (b3s, b3d)):
        nc.sync.dma_start(t[:], d.ap())
    for dst, srct in ((w1s, w1f), (w2s, w2f), (w3s, w3f), (i128, i1f)):
        nc.vector.tensor_copy(dst[:], srct[:])

    xpool = ctx.enter_context(tc.tile_pool(name="xin", bufs=2))
    xtpool = ctx.enter_context(tc.tile_pool(name="xts", bufs=2))
    h1pool = ctx.enter_context(tc.tile_pool(name="h1s", bufs=2))
    h2pool = ctx.enter_context(tc.tile_pool(name="h2s", bufs=2))
    h3pool = ctx.enter_context(tc.tile_pool(name="h3s", bufs=2))
    opool = ctx.enter_context(tc.tile_pool(name="obuf", bufs=2))
    tinp = ctx.enter_context(tc.tile_pool(name="tinp", bufs=2, space="PSUM"))
    mmp = ctx.enter_context(tc.tile_pool(name="mmp", bufs=3, space="PSUM"))

    for c in [i for _ in range(reps) for i in range(NCH)]:
        # ---- load [128, 2048] contiguous (64 padded rows per partition)
        xin = xpool.tile([128, CH_SUP * 16 * FP], F32R, tag="xin")
        src = AP(ev, c * CHUNK * FP, [[CH_SUP * 16 * FP, 128],
                                      [1, CH_SUP * 16 * FP]])
        nc.gpsimd.dma_start(xin[:], src)
        obuf = opool.tile([128, CH_SUP * 16 * FOUT], F32, tag="obuf")

        for s in range(CH_SUP):
            xv = xin[:, 512 * s:512 * (s + 1)]
            ov = obuf[:, 1040 * s:1040 * (s + 1)]

            # ---- transpose in: 4 x ([128, 128] window -> [128, 128])
            tpsum = tinp.tile([128, 512], F32R, tag="tps")
            for t in range(4):
                nc.tensor.transpose(tpsum[:, 128 * t:128 * (t + 1)],
                                    xv[:, 128 * t:128 * (t + 1)], i128[:])

            xts = xtpool.tile([128, 512], F32R, tag="xts")
            nc.scalar.copy(xts[:], tpsum[:])

            # ---- L1: blockdiag pairs, K=49, N=512
            ps1 = mmp.tile([128, 1024], F32, tag="mm")
            nc.tensor.matmul(ps1[:, 0:512], w1s[0:49, :], xts[0:49, :],
                             start=True, stop=True)
            nc.tensor.matmul(ps1[:, 512:1024], w1s[64:113, :], xts[64:113, :],
                             start=True, stop=True)
            h1s = h1pool.tile([128, 1024], F32R, tag="h1s")
            nc.scalar.activation(h1s[:], ps1[:], AF.Relu, bias=b1s[:])

            # ---- L2
            ps2 = mmp.tile([128, 1024], F32, tag="mm")
            nc.tensor.matmul(ps2[:, 0:512], w2s[:], h1s[:, 0:512],
                             start=True, stop=True)
            nc.tensor.matmul(ps2[:, 512:1024], w2s[:], h1s[:, 512:1024],
                             start=True, stop=True)
            h2s = h2pool.tile([128, 1024], F32R, tag="h2s")
            nc.vector.tensor_scalar(h2s[:], ps2[:], b2s[:], 0.0,
                                    ALU.add, ALU.max)

            # ---- L3
            ps3 = mmp.tile([128, 1024], F32, tag="mm")
            nc.tensor.matmul(ps3[:, 0:512], w3s[:], h2s[:, 0:512],
                             start=True, stop=True)
            nc.tensor.matmul(ps3[:, 512:1024], w3s[:], h2s[:, 512:1024],
                             start=True, stop=True)
            h3s = h3pool.tile([128, 1024], F32R, tag="h3s")
            nc.scalar.activation(h3s[:], ps3[:], AF.Relu, bias=b3s[:])

            # ---- transpose out: 8 x [128,128] -> particle-major pairs
            psb = mmp.tile([128, 1024], F32R, tag="mm")
            for u in range(8):
                nc.tensor.transpose(psb[:, 128 * u:128 * (u + 1)],
                                    h3s[:, 128 * u:128 * (u + 1)], i128[:])

            # ---- masked copy into output buffer + mask column
            # column groups: half h=0 -> blocks (4t, 4t+1); h=1 -> (4t+2, 4t+3)
            for h in range(2):
                dst = _inject(ov[:, 130 * h:130 * h + 64], [[260, 4], [65, 2]])
                msk = xv[:, 16 + 64 * h:16 + 64 * h + 1]
                msk = AP(msk.tensor, msk.offset,
                         [list(msk.ap[0]), [128, 4], [32, 2], [0, 64]])
                nc.vector.tensor_tensor(dst, psb[:, 512 * h:512 * (h + 1)],
                                        msk, ALU.mult)
            mcol_dst = _inject(ov[:, 64:65], [[65, 16]])
            mcol_src = _inject(xv[:, 16:17], [[32, 16]])
            nc.vector.tensor_copy(mcol_dst, mcol_src)

        dst = AP(out, c * CHUNK * FOUT, [[CH_SUP * 16 * FOUT, 128],
                                         [1, CH_SUP * 16 * FOUT]])
        nc.sync.dma_start(dst, obuf[:])

    ctx.close()


def kernel(events, W1, b1, W2, b2, W3, b3):
    global LAST_RESULTS
    events = np.ascontiguousarray(np.asarray(events, dtype=np.float32))
    key = "nc"
    if key not in _CACHE:
        _CACHE[key] = _build((np.asarray(W1, np.float32), np.asarray(b1, np.float32),
                              np.asarray(W2, np.float32), np.asarray(b2, np.float32),
                              np.asarray(W3, np.float32), np.asarray(b3, np.float32)))
    nc = _CACHE[key]

    flat = events.reshape(B * P, F)
    padded = np.zeros((B * P, FP), dtype=np.float32)
    padded[:, :F] = flat
    in_maps = [{"events": np.ascontiguousarray(padded[c * NPART:(c + 1) * NPART])}
               for c in range(NCORES)]
    res = run_bass_kernel_spmd(nc, in_maps, core_ids=list(range(NCORES)),
                               trace=TRACE)
    LAST_RESULTS = res
    out = np.concatenate([res.results[c]["out"] for c in range(NCORES)], axis=0)
    return out.reshape(B, P, FOUT)
